# revision 20
# baseline (speedup 1.0000x reference)
"""CPC (contrastive predictive coding) loss on 8 Trainium2 NeuronCores.

Problem: loss = mean over (t, k, i) of cross_entropy(scores[t,k,i,:], i) with
scores[t,k,i,j] = <c_proj[i,t], z[j,t+k]> / TEMP,  c_proj = c_seq @ W + b,
t in [0, Tm), k in [1, H], i,j in [0, B).

Distribution: sequence-parallel over anchor time t.  Every core runs an
identical program over TSLOT=14 anchor slots (7 "pair tiles" of 2 consecutive
anchors each); cores with fewer real anchors carry zero-padded slots whose
contributions are removed by per-core validity masks.  Each core returns a
(128,1) vector of partial sums; the host adds them up and divides by the term
count.

v2 design (vs the bf16 v1): 1/TEMP folded into W and b on the host, all
matmuls fp8(e4m3) with DoubleRow perf mode (K=256/instruction, 0.5 cyc/row).
Each pair tile is split into two PSUM half-tiles (16/15 shift groups, 2 banks
each, 3-deep rotation) so the PSUM recycle chain never gates the matmuls.
Softmax work is balanced across engines per half:
  scores (PE) -> pairwise j-max (Pool stt, bf16 out) -> grouped reduce_max
  (DVE, negated) -> broadcast subtract (A-half DVE / B-half Pool) -> exp
  (ACT, bf16) -> pairwise j-add (A DVE / B Pool) -> grouped reduce (DVE)
  -> batched Ln at the end (ACT; table prefetched during the last tiles).
Positive terms: banded Gram matmuls (4 batch elements per PSUM quadrant x 16
column blocks in one dedicated 2-bank psum tile), emitted right after the
first two score tiles to keep PE continuously busy, masked/accumulated by a
single Pool stt.  Host fp8-emulation error vs fp32 reference: ~1.6e-3 rel.
"""

import numpy as np
import ml_dtypes

B, T, D = 64, 128, 512
H = 30
TEMP = 0.07
NCORE = 8
TSLOT = 14            # padded anchor slots per core -> 7 pair tiles
NPAIR = TSLOT // 2
TS = TSLOT - 1 + H    # 43 z timesteps per core (slab + horizon halo)
G = H + 1             # 31 shift groups per pair tile
GA = 16               # groups in half-tile A (B gets G-GA=15)
TM = T - H            # 98 real anchors
NQ = 2                # DoubleRow contraction chunks (K=256 each)
NACC = 17             # acc: [0]=logS [1..8)=res [8]=pos1 [9..16)=rowC [16]=pos2

_REAL = [13, 13, 12, 12, 12, 12, 12, 12]
_T0 = [0, 13, 26, 38, 50, 62, 74, 86]

_CACHE = {}


def _build_program(loop_n=None, variant="full"):
    import concourse.bass as bass
    import concourse.bacc as bacc
    import concourse.tile as tile
    import concourse.mybir as mybir
    from contextlib import ExitStack

    dt = mybir.dt
    AF = mybir.ActivationFunctionType
    ALU = mybir.AluOpType
    AX = mybir.AxisListType
    DR = mybir.MatmulPerfMode.DoubleRow

    nc = bacc.Bacc("TRN2", debug=False, target_bir_lowering=False,
                   num_devices=NCORE)

    z_d = nc.dram_tensor("z8", [D, TS * B], dt.float8e4, kind="ExternalInput").ap()
    c_d = nc.dram_tensor("c8", [D, TSLOT * B], dt.float8e4, kind="ExternalInput").ap()
    w_d = nc.dram_tensor("w8", [D, D], dt.float8e4, kind="ExternalInput").ap()
    b_d = nc.dram_tensor("b_f", [D], dt.float32, kind="ExternalInput").ap()
    vm_d = nc.dram_tensor("vm", [128, NPAIR * G + NPAIR], dt.float32, kind="ExternalInput").ap()
    bd_d = nc.dram_tensor("band", [128, 16 * TS + 128], dt.float32, kind="ExternalInput").ap()
    ind_d = nc.dram_tensor("ind8", [G, 1984], dt.bfloat16, kind="ExternalInput").ap()
    out_d = nc.dram_tensor("partial", [128, 1], dt.float32, kind="ExternalOutput").ap()

    HALVES = ((0, GA), (GA, G - GA))          # (g0, gn) for the two halves

    with tile.TileContext(nc) as tc, ExitStack() as ctx:
        con = ctx.enter_context(tc.tile_pool(name="con", bufs=1))
        wrk = ctx.enter_context(tc.tile_pool(name="wrk", bufs=6))

        def _body():
            # ---------------- input loads ----------------
            # One shared DMA device: order so c_proj's inputs (b,w,c) land
            # first, then z (first halves of each row chunk first), masks last.
            b_sb = con.tile([128, 4], dt.float32, tag="b", name="b_sb")
            nc.sync.dma_start(b_sb[:], b_d.rearrange("(c p) -> p c", p=128))
            wt_a = con.tile([128, 4, D], dt.float8e4, tag="w", name="wt_a")
            nc.sync.dma_start(wt_a[:], w_d.rearrange("(k p) n -> p k n", p=128))
            ct_a = con.tile([128, 4, TSLOT * B], dt.float8e4, tag="c", name="ct_a")
            nc.sync.dma_start(ct_a[:], c_d.rearrange("(k p) n -> p k n", p=128))
            zt_a = con.tile([128, 4, TS * B], dt.float8e4, tag="z", name="zt_a")
            nc.sync.dma_start(zt_a[:], z_d.rearrange("(k p) n -> p k n", p=128))
            wt = [wt_a[:, 0:2, :], wt_a[:, 2:4, :]]
            ct = [ct_a[:, 0:2, :], ct_a[:, 2:4, :]]
            zt = [zt_a[:, 0:2, :], zt_a[:, 2:4, :]]
            vm_sb = con.tile([128, NPAIR * G + NPAIR], dt.float32, tag="vm", name="vm_sb")
            nc.sync.dma_start(vm_sb[:], vm_d)
            bd_sb = con.tile([128, 16 * TS + 128], dt.float32, tag="bd", name="bd_sb")
            nc.sync.dma_start(bd_sb[:], bd_d)
            ind_sb = con.tile([G, 1984], dt.bfloat16, tag="ind", name="ind_sb")
            nc.sync.dma_start(ind_sb[:], ind_d)

            acc = con.tile([128, NACC], dt.float32, tag="acc", name="acc")
            nc.vector.memset(acc[:], 0.0)
            if variant == "dmaonly":
                for q in range(NQ):
                    nc.vector.tensor_reduce(acc[:, 0:1], zt_a[:, 2 * q, 0:64],
                                            axis=AX.X, op=ALU.add)
                    nc.vector.tensor_reduce(acc[:, 1:2], ct_a[:, 2 * q, 0:64],
                                            axis=AX.X, op=ALU.add)
                    nc.vector.tensor_reduce(acc[:, 2:3], wt_a[:, 2 * q, 0:64],
                                            axis=AX.X, op=ALU.add)
            s_all = con.tile([128, NPAIR * G], dt.bfloat16, tag="sall", name="s_all")

            # ---------------- c_projT (fp8, (d_out, (t, i))) ------------
            cq = []
            for q in range(NQ):
                cqt = con.tile([128, 2, 32 * B], dt.float8e4, tag=f"cq{q}",
                               name=f"cq{q}")
                for kt in range(2):
                    nc.gpsimd.memset(cqt[:, kt, TSLOT * B:32 * B], 0.0)
                cq.append(cqt)
            with tc.tile_pool(name="pcp", bufs=2, space="PSUM") as pcp:
                for m in range(4 if variant != "dmaonly" else 0):
                    psc = pcp.tile([128, TSLOT * B], dt.float32, tag="psc",
                                   name="psc")
                    for n0 in (0, 512):
                        nn = min(512, TSLOT * B - n0)
                        for q in range(NQ):
                            nc.tensor.matmul(
                                psc[:, n0:n0 + nn],
                                wt[q][:, :, m * 128:(m + 1) * 128],
                                ct[q][:, :, n0:n0 + nn],
                                start=(q == 0), stop=(q == NQ - 1),
                                perf_mode=DR)
                    nc.scalar.activation(cq[m // 2][:, m % 2, 0:TSLOT * B], psc[:],
                                         AF.Identity, bias=b_sb[:, m:m + 1])

            # ---------------- pair tiles (half-tiles) + pos ----------------
            # GPSIMD cannot touch PSUM, so PSUM-side work is DVE-only:
            #   SX(p): score matmuls (PE) -> grouped reduce_max (DVE, PSUM) ->
            #          rowmax split -C (DVE) -> fp8 residual -r~/8 (ACT, exact
            #          f8 grid) -> dequant (DVE) + accums (Pool)
            #   Y(p):  PE transpose of the residual row + ACT quant -> fp8
            #          DoubleRow indicator matmul adds -r~ into PSUM -> exp
            #          (ACT, bias=-C) straight out of PSUM
            #   Z(p):  pairwise j-sums (Pool, SBUF bf16) -> grouped reduce
            #          tail (DVE)
            # Stages are emitted 3 deep so no engine stalls on its own pair.
            onez = con.tile([128, 1], dt.float32, tag="onez", name="onez")
            nc.vector.memset(onez[:], 1.0)
            negT8 = con.tile([32, 128], dt.bfloat16, tag="nT8",
                             name="negT8")
            ident = bd_sb[:, 16 * TS:16 * TS + 128]
            pos_pool = ctx.enter_context(
                tc.tile_pool(name="ppo", bufs=1, space="PSUM"))
            with tc.tile_pool(name="pps", bufs=3, space="PSUM") as pps:

                def emit_pos():
                    # 43-col blocks packed 11 per 512-fp32 psum bank so no
                    # matmul output crosses a bank boundary
                    pspos = pos_pool.tile([128, 1024], dt.float32, tag="pp",
                                          name="pspos")
                    for i in range(B):
                        jq, ii = i % 4, i // 4
                        col = (ii // 11) * 512 + (ii % 11) * TS
                        for q in range(NQ):
                            for kt in range(2):
                                c4 = cq[q][:, kt, :].rearrange(
                                    "p (t i) -> p t i", i=B)[:, :, i]
                                z4 = zt[q][:, kt, :].rearrange(
                                    "p (s i) -> p s i", i=B)[:, :, i]
                                nc.tensor.matmul(
                                    pspos[32 * jq:32 * jq + 32, col:col + TS],
                                    c4, z4,
                                    start=(q == 0 and kt == 0),
                                    stop=(q == NQ - 1 and kt == 1),
                                    tile_position=(0, 32 * jq),
                                    skip_group_check=True)
                    for seg, (c0, nb, a0) in enumerate(
                            ((0, 11, 8), (512, 5, 16))):
                        junk3 = wrk.tile([128, nb * TS], dt.float32,
                                         tag=f"junk3{seg}", name="junk3")
                        nc.vector.scalar_tensor_tensor(
                            junk3[:], pspos[:, c0:c0 + nb * TS], -1.0,
                            bd_sb[:, (11 * TS if seg else 0):
                                  (11 * TS if seg else 0) + nb * TS],
                            op0=ALU.mult, op1=ALU.mult,
                            accum_out=acc[:, a0:a0 + 1])

                def emit_SX(p):
                    halves = []
                    mx1 = wrk.tile([128, G], dt.float32, tag="mx1", name="mx1")
                    for g0, gn in HALVES:
                        ph = pps.tile([128, GA * B], dt.float32, tag="ps",
                                      name=f"ps{p}_{g0}")
                        for n0 in range(0, gn * B, 512):
                            nn = min(512, gn * B - n0)
                            c0 = (2 * p + g0) * B + n0
                            for q in range(NQ):
                                nc.tensor.matmul(
                                    ph[:, n0:n0 + nn],
                                    cq[q][:, :, (2 * p) * B:(2 * p) * B + 128],
                                    zt[q][:, :, c0:c0 + nn],
                                    start=(q == 0), stop=(q == NQ - 1),
                                    perf_mode=DR)
                        halves.append(ph)
                        ph3 = ph[:, 0:gn * B].rearrange("p (g j) -> p g j", j=B)
                        nc.vector.tensor_reduce(mx1[:, g0:g0 + gn], ph3,
                                                axis=AX.X, op=ALU.max,
                                                negate=True)
                    # rowmax split: -C = min_g(-max); residual -r~/8 on the f8
                    # grid (quantize on ACT, dequant on DVE for consistency)
                    negC = wrk.tile([128, 1], dt.float32, tag="negC",
                                    name="negC")
                    nc.vector.tensor_reduce(negC[:], mx1[:], axis=AX.X,
                                            op=ALU.min)
                    negrb = wrk.tile([128, G], dt.bfloat16, tag="negrb",
                                     name="negrb")
                    nc.vector.scalar_tensor_tensor(
                        negrb[:], mx1[:], 1.0,
                        negC[:].broadcast_to((128, G)),
                        op0=ALU.mult, op1=ALU.subtract)
                    rq32 = wrk.tile([128, 32], dt.float32, tag="rq32",
                                    name="rq32")
                    nc.scalar.copy(rq32[:, 0:G], negrb[:])
                    nc.vector.memset(rq32[:, G:32], 0.0)
                    junk2 = wrk.tile([128, G], dt.float32, tag="junk2",
                                     name="junk2")
                    nc.vector.scalar_tensor_tensor(
                        junk2[:], negrb[:], -1.0,
                        vm_sb[:, p * G:(p + 1) * G],
                        op0=ALU.mult, op1=ALU.mult,
                        accum_out=acc[:, 1 + p:2 + p])
                    junkc2 = wrk.tile([128, 1], dt.float32, tag="junkc2",
                                      name="junkc2")
                    nc.vector.scalar_tensor_tensor(
                        junkc2[:], negC[:], -1.0,
                        vm_sb[:, NPAIR * G + p:NPAIR * G + p + 1],
                        op0=ALU.mult, op1=ALU.mult,
                        accum_out=acc[:, 9 + p:10 + p])
                    return {"halves": halves, "negC": negC, "rq32": rq32}

                def emit_Y(p, st):
                    ptr = pos_pool.tile([32, 128], dt.float32, tag="pp",
                                        name="ptr")
                    nc.tensor.transpose(ptr[:], st["rq32"][:], ident)
                    nc.scalar.activation(negT8[:], ptr[:], AF.Identity)
                    esbs = []
                    for (g0, gn), ph in zip(HALVES, st["halves"]):
                        for n0 in range(0, gn * B, 512):
                            nn = min(512, gn * B - n0)
                            nc.tensor.matmul(
                                ph[:, n0:n0 + nn], negT8[0:G, :],
                                ind_sb[0:G, g0 * B + n0:g0 * B + n0 + nn],
                                start=False, stop=True,
                                skip_group_check=True)
                        esb = wrk.tile([128, gn * B], dt.bfloat16,
                                       tag=f"esb{g0}", name="esb")
                        nc.scalar.activation(esb[:], ph[:, 0:gn * B],
                                             AF.Exp, bias=st["negC"][:])
                        esbs.append(esb)
                    st["esbs"] = esbs
                    return esbs[-1]

                def emit_Z(p, st):
                    for (g0, gn), esb in zip(HALVES, st["esbs"]):
                        e3 = esb[:].rearrange("p (g j) -> p g j", j=B)
                        with nc.allow_low_precision("bf16 group sums"):
                            nc.vector.tensor_reduce(
                                s_all[:, p * G + g0:p * G + g0 + gn], e3,
                                axis=AX.X, op=ALU.add)

                npair = 0 if variant == "dmaonly" else NPAIR
                stages = {}
                last_esb = None
                for k in range(npair + 2):
                    if k < npair:
                        if variant == "noce":
                            st = {"halves": []}
                            for g0, gn in HALVES:
                                ph = pps.tile([128, GA * B], dt.float32,
                                              tag="ps", name=f"ps{k}_{g0}")
                                for n0 in range(0, gn * B, 512):
                                    nn = min(512, gn * B - n0)
                                    c0 = (2 * k + g0) * B + n0
                                    for q in range(NQ):
                                        nc.tensor.matmul(
                                            ph[:, n0:n0 + nn],
                                            cq[q][:, :, (2 * k) * B:
                                                  (2 * k) * B + 128],
                                            zt[q][:, :, c0:c0 + nn],
                                            start=(q == 0),
                                            stop=(q == NQ - 1),
                                            perf_mode=DR)
                                st["halves"].append(ph)
                            junkc = wrk.tile([128, 1], dt.float32, tag="junkc",
                                             name="junkc")
                            nc.vector.tensor_reduce(
                                junkc[:], st["halves"][0][:, 0:B],
                                axis=AX.X, op=ALU.add)
                            continue
                        stages[k] = emit_SX(k)
                    if variant != "full":
                        continue
                    if 0 <= k - 1 < npair:
                        last_esb = emit_Y(k - 1, stages[k - 1])
                        if k - 1 == npair - 1 and last_esb is not None:
                            # prefetch the Ln table right after the last Exp
                            lnp = wrk.tile([128, 1], dt.float32, tag="lnp",
                                           name="lnpre")
                            nc.scalar.activation(lnp[:], last_esb[:, 0:1],
                                                 AF.Ln)
                    if 0 <= k - 2 < npair:
                        emit_Z(k - 2, stages.pop(k - 2))
                if variant == "full":
                    emit_pos()

            if variant == "full":
                logs = con.tile([128, NPAIR * G], dt.float32, tag="logs",
                                name="logs")
                nc.scalar.activation(logs[:], s_all[:], AF.Ln)
                junkl = con.tile([128, NPAIR * G], dt.float32, tag="junkl",
                                 name="junkl")
                nc.vector.scalar_tensor_tensor(
                    junkl[:], logs[:], 1.0, vm_sb[:, 0:NPAIR * G],
                    op0=ALU.mult, op1=ALU.mult, accum_out=acc[:, 0:1])
            part = con.tile([128, 1], dt.float32, tag="part", name="part")
            nc.vector.tensor_reduce(part[:], acc[:], axis=AX.X, op=ALU.add)
            nc.sync.dma_start(out_d, part[:])

        if loop_n:
            with tc.For_i(0, loop_n, 1):
                _body()
        else:
            _body()

    nc.compile()
    return nc


def get_program(loop_n=None, variant="full"):
    key = ("nc", loop_n, variant)
    if key not in _CACHE:
        _CACHE[key] = _build_program(loop_n, variant)
    return _CACHE[key]


def make_core_inputs(m, z, c, W, b):
    """Host-side sharding, 1/TEMP folding and fp8 cast for core m."""
    f8 = ml_dtypes.float8_e4m3
    t0, nreal = _T0[m], _REAL[m]

    s_lo = t0 + 1
    n_avail = min(TS, T - s_lo)
    z8 = np.zeros((D, TS, B), dtype=f8)
    z8[:, :n_avail] = z[:, s_lo:s_lo + n_avail].astype(f8).transpose(2, 1, 0)

    c8 = np.zeros((D, TSLOT, B), dtype=f8)
    c8[:, :nreal] = c[:, t0:t0 + nreal].astype(f8).transpose(2, 1, 0)

    # pair-tile validity: partition p = half*64 + i, half anchored at t+half
    p_idx = np.arange(128)
    g_idx = np.arange(G)
    th = p_idx[:, None, None] // B                     # (128,1,1)
    pp = np.arange(NPAIR)[None, :, None]               # (1,7,1)
    gg = g_idx[None, None, :]                          # (1,1,31)
    slot = 2 * pp + th
    gvalid = np.where(th == 0, gg <= H - 1, (gg >= 1) & (gg <= H))
    vm = ((slot < nreal) & gvalid).astype(np.float32).reshape(128, NPAIR * G)

    # pos band mask: partition p = 32*jq + slot, column block ii, col s;
    # valid iff slot is a real anchor and s in [slot, slot+H)
    slot2 = (p_idx % 32)[:, None]                      # (128,1)
    si = np.arange(TS)[None, :]                        # (1,43)
    band = ((slot2 < nreal) & (si >= slot2)
            & (si < slot2 + H)).astype(np.float32)
    bd16 = np.tile(band, (1, 16))

    vmrow = vm.reshape(128, NPAIR, G).sum(axis=2).astype(np.float32)
    bd_id = np.concatenate([bd16, np.eye(128, dtype=np.float32)], axis=1)

    # bf16 residual indicator: 1.0 on the block diagonal
    ind8 = np.zeros((G, 1984), dtype=ml_dtypes.bfloat16)
    for g in range(G):
        col0 = g * B if g < GA else 1024 + (g - GA) * B
        ind8[g, col0:col0 + B] = 1.0

    return {
        "z8": z8.reshape(D, TS * B),
        "c8": c8.reshape(D, TSLOT * B),
        "w8": (W / TEMP).astype(f8),
        "b_f": (b / TEMP).astype(np.float32),
        "vm": np.concatenate([vm, vmrow], axis=1),
        "band": bd_id,
        "ind8": ind8,
    }


def kernel(z_seq, c_seq, W_cpc, b_cpc):
    z = np.asarray(z_seq, dtype=np.float32)
    c = np.asarray(c_seq, dtype=np.float32)
    W = np.asarray(W_cpc, dtype=np.float32)
    b = np.asarray(b_cpc, dtype=np.float32)

    nc = get_program()
    in_maps = [make_core_inputs(m, z, c, W, b) for m in range(NCORE)]

    from concourse.bass_utils import run_bass_kernel_spmd
    res = run_bass_kernel_spmd(nc, in_maps, core_ids=list(range(NCORE)))

    tot = sum(float(r["partial"].astype(np.float64).sum()) for r in res.results)
    return np.float32(tot / (TM * H * B))


if __name__ == "__main__":
    rng = np.random.default_rng(0)
    out = kernel(
        rng.standard_normal((B, T, D), dtype=np.float32),
        rng.standard_normal((B, T, D), dtype=np.float32),
        (rng.standard_normal((D, D)) / np.sqrt(D)).astype(np.float32),
        (rng.standard_normal(D) * 0.01).astype(np.float32),
    )
    print("loss:", out)


# revision 23
# speedup vs baseline: 1.4803x; 1.4803x over previous
"""CPC (contrastive predictive coding) loss on 8 Trainium2 NeuronCores.

Problem: loss = mean over (t, k, i) of cross_entropy(scores[t,k,i,:], i) with
scores[t,k,i,j] = <c_proj[i,t], z[j,t+k]> / TEMP,  c_proj = c_seq @ W + b,
t in [0, Tm), k in [1, H], i,j in [0, B).

Distribution: sequence-parallel over anchor time t.  Every core runs an
identical program over TSLOT=14 anchor slots (7 "pair tiles" of 2 consecutive
anchors each); cores with fewer real anchors carry zero-padded slots whose
contributions are removed by per-core validity masks.  Each core returns a
(128,1) vector of partial sums; the host adds them up and divides by the term
count.

v3 design (1/TEMP folded into W,b on the host):
- All score/proj matmuls run fp8(e4m3) with DoubleRow perf mode (K=256 per
  instruction, 0.5 cyc/row), chunked to 512 output columns (PSUM bank limit).
- Pair tiles are split into two 2-bank PSUM half-tiles (16/15 shift groups,
  3-deep rotation).  Per pair: grouped reduce_max on DVE straight from PSUM
  (bf16 negated output = the subtracted value, so add-back is consistent),
  dequant row (ACT) -> PE transpose -> bf16 quant (ACT, exact) -> a bf16
  "indicator" matmul adds -max into PSUM on the PE -> exp (ACT, bf16 out,
  both halves into one tile) -> one grouped reduce_sum (DVE) -> batched Ln
  at the end (table prefetched behind the last exp).
- GPSIMD cannot touch PSUM and its generic tensor ops do not exist in
  codegen, so elementwise work lives on DVE/ACT/PE only.
- Positive terms: banded Gram matmuls into a dedicated psum tile (43-col
  blocks packed 11 per bank), masked/accumulated by two DVE stts.
- Inputs arrive as 3 merged DMA blobs (fp8 z|c|w, fp32 b|vm|band|identity,
  bf16 indicator); the loop body is unrolled 2x with ping-pong input
  buffers so each iteration's DMA hides under the previous compute.
Host fp8-emulation error vs the fp32 reference: ~1.6e-3 relative.
"""

import numpy as np
import ml_dtypes

B, T, D = 64, 128, 512
H = 30
TEMP = 0.07
NCORE = 8
TSLOT = 14            # padded anchor slots per core -> 7 pair tiles
NPAIR = TSLOT // 2
TS = TSLOT - 1 + H    # 43 z timesteps per core (slab + horizon halo)
G = H + 1             # 31 shift groups per pair tile
GA = 16               # groups in half-tile A (B half gets 15)
TM = T - H            # 98 real anchors
NQ = 2                # DoubleRow contraction chunks (K=256 each)
NACC = 10             # acc: [0]=logS, [1..8)=max, [8]=pos1, [9]=pos2

ZB = 4 * TS * B       # fp8 blob layout: z | c | w  (per partition)
CB = 4 * TSLOT * B
WB = 4 * D
F8B = ZB + CB + WB
CST = 4 + NPAIR * G + 16 * TS + 128   # fp32 blob: b | vm | band | identity

_REAL = [13, 13, 12, 12, 12, 12, 12, 12]
_T0 = [0, 13, 26, 38, 50, 62, 74, 86]

_CACHE = {}


def _build_program(loop_n=None, variant="full"):
    import concourse.bass as bass
    import concourse.bacc as bacc
    import concourse.tile as tile
    import concourse.mybir as mybir
    from contextlib import ExitStack

    dt = mybir.dt
    AF = mybir.ActivationFunctionType
    ALU = mybir.AluOpType
    AX = mybir.AxisListType
    DR = mybir.MatmulPerfMode.DoubleRow

    nc = bacc.Bacc("TRN2", debug=False, target_bir_lowering=False,
                   num_devices=NCORE)

    f8_d = nc.dram_tensor("f8in", [128, F8B], dt.float8e4, kind="ExternalInput").ap()
    cst_d = nc.dram_tensor("cst", [128, CST], dt.float32, kind="ExternalInput").ap()
    ind_d = nc.dram_tensor("ind8", [G, 1984], dt.bfloat16, kind="ExternalInput").ap()
    out_d = nc.dram_tensor("partial", [128, 1], dt.float32, kind="ExternalOutput").ap()

    HALVES = ((0, GA), (GA, G - GA))

    with tile.TileContext(nc) as tc, ExitStack() as ctx:
        con = ctx.enter_context(tc.tile_pool(name="con", bufs=1))
        io = ctx.enter_context(tc.tile_pool(name="io", bufs=1))
        wrk = ctx.enter_context(tc.tile_pool(name="wrk", bufs=6))
        pos_pool = ctx.enter_context(
            tc.tile_pool(name="ppo", bufs=1, space="PSUM"))

        def _body(ph):
            # ---------------- input loads (3 DMAs) ----------------
            f8t = io.tile([128, F8B], dt.float8e4, tag=f"f8{ph}", name="f8t")
            nc.sync.dma_start(f8t[:], f8_d)
            cst = io.tile([128, CST], dt.float32, tag=f"cst{ph}", name="cst")
            nc.sync.dma_start(cst[:], cst_d)
            ind_sb = io.tile([G, 1984], dt.bfloat16, tag=f"ind{ph}",
                             name="ind_sb")
            nc.sync.dma_start(ind_sb[:], ind_d)

            zt_a = f8t[:, 0:ZB].rearrange("p (k n) -> p k n", k=4)
            ct_a = f8t[:, ZB:ZB + CB].rearrange("p (k n) -> p k n", k=4)
            wt_a = f8t[:, ZB + CB:F8B].rearrange("p (k n) -> p k n", k=4)
            zt = [zt_a[:, 0:2, :], zt_a[:, 2:4, :]]
            ct = [ct_a[:, 0:2, :], ct_a[:, 2:4, :]]
            wt = [wt_a[:, 0:2, :], wt_a[:, 2:4, :]]
            b_sb = cst[:, 0:4]
            vm_sb = cst[:, 4:4 + NPAIR * G]
            bd_sb = cst[:, 4 + NPAIR * G:4 + NPAIR * G + 16 * TS]
            ident = cst[:, 4 + NPAIR * G + 16 * TS:CST]

            acc = io.tile([128, NACC], dt.float32, tag=f"acc{ph}", name="acc")
            nc.vector.memset(acc[:], 0.0)
            if variant == "dmaonly":
                nc.vector.tensor_reduce(acc[:, 0:1], cst[:, 4:68],
                                        axis=AX.X, op=ALU.add)
                nc.vector.tensor_reduce(acc[:, 1:2], ind_sb[:, 0:64],
                                        axis=AX.X, op=ALU.add)
            s_all = io.tile([128, NPAIR * G], dt.bfloat16, tag=f"sa{ph}",
                            name="s_all")

            # ---------------- c_projT (fp8, (d_out, (t, i))) ------------
            cq = []
            for q in range(NQ):
                cqt = io.tile([128, 2, 32 * B], dt.float8e4, tag=f"cq{q}{ph}",
                              name=f"cq{q}")
                for kt in range(2):
                    nc.gpsimd.memset(cqt[:, kt, TSLOT * B:32 * B], 0.0)
                cq.append(cqt)
            with tc.tile_pool(name=f"pcp{ph}", bufs=2, space="PSUM") as pcp:
                for m in range(4 if variant != "dmaonly" else 0):
                    psc = pcp.tile([128, TSLOT * B], dt.float32, tag="psc",
                                   name="psc")
                    for n0 in (0, 512):
                        nn = min(512, TSLOT * B - n0)
                        for q in range(NQ):
                            nc.tensor.matmul(
                                psc[:, n0:n0 + nn],
                                wt[q][:, :, m * 128:(m + 1) * 128],
                                ct[q][:, :, n0:n0 + nn],
                                start=(q == 0), stop=(q == NQ - 1),
                                perf_mode=DR)
                    nc.scalar.activation(cq[m // 2][:, m % 2, 0:TSLOT * B],
                                         psc[:], AF.Identity,
                                         bias=b_sb[:, m:m + 1])

            # ---------------- pair tiles (half-tiles) + pos ----------------
            with tc.tile_pool(name=f"pps{ph}", bufs=3, space="PSUM") as pps:

                def emit_pos():
                    # 43-col blocks packed 11 per 512-fp32 psum bank
                    pspos = pos_pool.tile([128, 1024], dt.float32, tag="pp",
                                          name="pspos")
                    for i in range(B):
                        jq, ii = i % 4, i // 4
                        col = (ii // 11) * 512 + (ii % 11) * TS
                        for q in range(NQ):
                            for kt in range(2):
                                c4 = cq[q][:, kt, :].rearrange(
                                    "p (t i) -> p t i", i=B)[:, :, i]
                                z4 = zt[q][:, kt, :].rearrange(
                                    "p (s i) -> p s i", i=B)[:, :, i]
                                nc.tensor.matmul(
                                    pspos[32 * jq:32 * jq + 32, col:col + TS],
                                    c4, z4,
                                    start=(q == 0 and kt == 0),
                                    stop=(q == NQ - 1 and kt == 1),
                                    tile_position=(0, 32 * jq),
                                    skip_group_check=True)
                    for seg, (c0, nb, a0) in enumerate(
                            ((0, 11, 8), (512, 5, 9))):
                        junk3 = wrk.tile([128, nb * TS], dt.float32,
                                         tag=f"junk3{seg}", name="junk3")
                        nc.vector.scalar_tensor_tensor(
                            junk3[:], pspos[:, c0:c0 + nb * TS], -1.0,
                            bd_sb[:, (11 * TS if seg else 0):
                                  (11 * TS if seg else 0) + nb * TS],
                            op0=ALU.mult, op1=ALU.mult,
                            accum_out=acc[:, a0:a0 + 1])

                def emit_scores(p):
                    halves = []
                    for g0, gn in HALVES:
                        ph_ = pps.tile([128, GA * B], dt.float32, tag="ps",
                                       name=f"ps{p}_{g0}")
                        for n0 in range(0, gn * B, 512):
                            nn = min(512, gn * B - n0)
                            c0 = (2 * p + g0) * B + n0
                            for q in range(NQ):
                                nc.tensor.matmul(
                                    ph_[:, n0:n0 + nn],
                                    cq[q][:, :, (2 * p) * B:(2 * p) * B + 128],
                                    zt[q][:, :, c0:c0 + nn],
                                    start=(q == 0), stop=(q == NQ - 1),
                                    perf_mode=DR)
                        halves.append(ph_)
                    return halves

                def emit_SX(p):
                    halves = emit_scores(p)
                    # negated grouped max in bf16: the bf16 value is both
                    # subtracted (via the indicator matmul) and added back
                    # (junk2), so the lse identity stays exact
                    mx1 = wrk.tile([128, G], dt.bfloat16, tag="mx1",
                                   name="mx1")
                    for (g0, gn), ph_ in zip(HALVES, halves):
                        ph3 = ph_[:, 0:gn * B].rearrange(
                            "p (g j) -> p g j", j=B)
                        nc.vector.tensor_reduce(mx1[:, g0:g0 + gn], ph3,
                                                axis=AX.X, op=ALU.max,
                                                negate=True)
                    rq32 = wrk.tile([128, G], dt.float32, tag="rq32",
                                    name="rq32")
                    nc.scalar.copy(rq32[:], mx1[:])
                    junk2 = wrk.tile([128, G], dt.float32, tag="junk2",
                                     name="junk2")
                    nc.vector.scalar_tensor_tensor(
                        junk2[:], mx1[:], -1.0, vm_sb[:, p * G:(p + 1) * G],
                        op0=ALU.mult, op1=ALU.mult,
                        accum_out=acc[:, 1 + p:2 + p])
                    return {"halves": halves, "rq32": rq32}

                def emit_Y(p, st):
                    ptr = pos_pool.tile([32, 128], dt.float32, tag="pp",
                                        name="ptr")
                    nc.tensor.transpose(ptr[0:G, :], st["rq32"][:], ident)
                    negT = con.tile([32, 128], dt.bfloat16, tag="nT",
                                    name="negT")
                    nc.scalar.activation(negT[0:G, :], ptr[0:G, :],
                                         AF.Identity)
                    esb = wrk.tile([128, G * B], dt.bfloat16, tag="esb",
                                   name="esb")
                    for (g0, gn), ph_ in zip(HALVES, st["halves"]):
                        for n0 in range(0, gn * B, 512):
                            nn = min(512, gn * B - n0)
                            nc.tensor.matmul(
                                ph_[:, n0:n0 + nn], negT[0:G, :],
                                ind_sb[0:G, g0 * B + n0:g0 * B + n0 + nn],
                                start=False, stop=True,
                                skip_group_check=True)
                        nc.scalar.activation(esb[:, g0 * B:(g0 + gn) * B],
                                             ph_[:, 0:gn * B], AF.Exp)
                    st["esb"] = esb
                    return esb

                def emit_Z(p, st):
                    e3 = st["esb"][:].rearrange("p (g j) -> p g j", j=B)
                    with nc.allow_low_precision("bf16 group sums"):
                        nc.vector.tensor_reduce(
                            s_all[:, p * G:(p + 1) * G], e3,
                            axis=AX.X, op=ALU.add)

                npair = 0 if variant == "dmaonly" else NPAIR
                softmax_on = variant in ("full", "nopos")
                stages = {}
                last_esb = None
                for k in range(npair + 2):
                    if k < npair:
                        if variant == "noce":
                            hh = emit_scores(k)
                            junkc = wrk.tile([128, 1], dt.float32, tag="junkc",
                                             name="junkc")
                            nc.vector.tensor_reduce(junkc[:], hh[0][:, 0:B],
                                                    axis=AX.X, op=ALU.add)
                            continue
                        stages[k] = emit_SX(k)
                    if not softmax_on:
                        continue
                    if 0 <= k - 1 < npair:
                        last_esb = emit_Y(k - 1, stages[k - 1])
                        if k - 1 == npair - 1 and last_esb is not None:
                            lnp = wrk.tile([128, 1], dt.float32, tag="lnp",
                                           name="lnpre")
                            nc.scalar.activation(lnp[:], last_esb[:, 0:1],
                                                 AF.Ln)
                    if 0 <= k - 2 < npair:
                        emit_Z(k - 2, stages.pop(k - 2))
                if variant == "full":
                    emit_pos()

            if softmax_on:
                logs = io.tile([128, NPAIR * G], dt.float32, tag=f"lg{ph}",
                               name="logs")
                nc.scalar.activation(logs[:], s_all[:], AF.Ln)
                junkl = wrk.tile([128, NPAIR * G], dt.float32, tag="junkl",
                                 name="junkl")
                nc.vector.scalar_tensor_tensor(
                    junkl[:], logs[:], 1.0, vm_sb[:],
                    op0=ALU.mult, op1=ALU.mult, accum_out=acc[:, 0:1])
            part = wrk.tile([128, 1], dt.float32, tag="part", name="part")
            nc.vector.tensor_reduce(part[:], acc[:], axis=AX.X, op=ALU.add)
            nc.sync.dma_start(out_d, part[:])

        if loop_n:
            assert loop_n % 2 == 0
            with tc.For_i(0, loop_n // 2, 1):
                _body(0)
                _body(1)
        else:
            _body(0)

    nc.compile()
    return nc


def get_program(loop_n=None, variant="full"):
    key = ("nc", loop_n, variant)
    if key not in _CACHE:
        _CACHE[key] = _build_program(loop_n, variant)
    return _CACHE[key]


def make_core_inputs(m, z, c, W, b):
    """Host-side sharding, 1/TEMP folding, fp8 cast, blob packing."""
    f8 = ml_dtypes.float8_e4m3
    bf = ml_dtypes.bfloat16
    t0, nreal = _T0[m], _REAL[m]

    s_lo = t0 + 1
    n_avail = min(TS, T - s_lo)
    z8 = np.zeros((D, TS, B), dtype=f8)
    z8[:, :n_avail] = z[:, s_lo:s_lo + n_avail].astype(f8).transpose(2, 1, 0)

    c8 = np.zeros((D, TSLOT, B), dtype=f8)
    c8[:, :nreal] = c[:, t0:t0 + nreal].astype(f8).transpose(2, 1, 0)

    w8 = (W / TEMP).astype(f8)

    # fp8 blob: per partition p the cols are z(4 chunks) | c(4) | w(4),
    # chunk k holding dram rows k*128+p
    blob = np.concatenate(
        [z8.reshape(4, 128, TS * B).transpose(1, 0, 2).reshape(128, ZB),
         c8.reshape(4, 128, TSLOT * B).transpose(1, 0, 2).reshape(128, CB),
         w8.reshape(4, 128, D).transpose(1, 0, 2).reshape(128, WB)], axis=1)

    # pair-tile validity: partition p = half*64 + i, half anchored at t+half
    p_idx = np.arange(128)
    g_idx = np.arange(G)
    th = p_idx[:, None, None] // B
    pp = np.arange(NPAIR)[None, :, None]
    gg = g_idx[None, None, :]
    slot = 2 * pp + th
    gvalid = np.where(th == 0, gg <= H - 1, (gg >= 1) & (gg <= H))
    vm = ((slot < nreal) & gvalid).astype(np.float32).reshape(128, NPAIR * G)

    # pos band mask: partition p = 32*jq + slot, column block ii, col s
    slot2 = (p_idx % 32)[:, None]
    si = np.arange(TS)[None, :]
    band = ((slot2 < nreal) & (si >= slot2)
            & (si < slot2 + H)).astype(np.float32)
    bd16 = np.tile(band, (1, 16))

    b_pad = (b / TEMP).astype(np.float32).reshape(4, 128).T.copy()
    cst = np.concatenate(
        [b_pad, vm, bd16, np.eye(128, dtype=np.float32)], axis=1)

    ind8 = np.zeros((G, 1984), dtype=bf)
    for g in range(G):
        col0 = g * B if g < GA else 1024 + (g - GA) * B
        ind8[g, col0:col0 + B] = 1.0

    return {"f8in": blob, "cst": cst, "ind8": ind8}


def kernel(z_seq, c_seq, W_cpc, b_cpc):
    z = np.asarray(z_seq, dtype=np.float32)
    c = np.asarray(c_seq, dtype=np.float32)
    W = np.asarray(W_cpc, dtype=np.float32)
    b = np.asarray(b_cpc, dtype=np.float32)

    nc = get_program()
    in_maps = [make_core_inputs(m, z, c, W, b) for m in range(NCORE)]

    from concourse.bass_utils import run_bass_kernel_spmd
    res = run_bass_kernel_spmd(nc, in_maps, core_ids=list(range(NCORE)))

    tot = sum(float(r["partial"].astype(np.float64).sum()) for r in res.results)
    return np.float32(tot / (TM * H * B))


if __name__ == "__main__":
    rng = np.random.default_rng(0)
    out = kernel(
        rng.standard_normal((B, T, D), dtype=np.float32),
        rng.standard_normal((B, T, D), dtype=np.float32),
        (rng.standard_normal((D, D)) / np.sqrt(D)).astype(np.float32),
        (rng.standard_normal(D) * 0.01).astype(np.float32),
    )
    print("loss:", out)
